# revision 14
# baseline (speedup 1.0000x reference)
"""Trainium2 Bass kernel for nn_MultiHeadAttention_57715770523709.

Reference semantics (B=2, F=2048, D=1024, H=16, DK=64):
    qp = (q @ Wq.T + bq)  -> [B,H,F,DK] (head-split)   (same k, v)
    scores = qp @ kp.T / sqrt(D)                        [B,H,F,F]
    attn = softmax(scores, axis=1)   # over the HEADS axis (quirk)
    res  = attn @ vp                                    [B,H,F,DK]
    res  = res.transpose(1,2,0,3).reshape(B,F,D)        # scrambles (h,f,b,d)
    out  = res @ W2.T + b2
    returns (out, attn)

Sharding: 8 cores; cores 0-3 own batch 0, cores 4-7 batch 1; each core owns a
512-wide slice of query positions with ALL heads resident, so the heads-axis
softmax is core-local.  The head-axis denominator is a full [fq,fk] map, so
attention is computed twice in the two layouts each consumer needs:
  pass B ([fk,fq], "transposed") feeds attn@v;  pass A ([fq,fk], natural)
produces the big attn output with DMA-friendly contiguous rows.
The permute(1,2,0,3) scramble is folded into the output projection by indexing
res with strided APs; sibling cores (same f-window, other batch) exchange res
via a pair AllGather because each scrambled output row mixes both batches.
All matmuls run in float32r (TF32-like single-pass mode; set USE_F32R = False
to fall back to exact-fp32 matmuls at 4x PE cost).
"""

import numpy as np

import concourse.mybir as mybir
import concourse.tile as tile
from concourse import bacc
from concourse.bass_utils import run_bass_kernel_spmd

B, F, D, H, DK = 2, 2048, 1024, 16, 64
N_CORES = 8
FQ = 512                      # query rows per core
SCALE = 1.0 / 32.0            # 1/sqrt(D)
USE_F32R = True

F32 = mybir.dt.float32
AF = mybir.ActivationFunctionType
ADD = mybir.AluOpType.add
MULT = mybir.AluOpType.mult

_CACHE = {}


def _denom_tree(nc, elem, estk):
    """Sum the 16 head slices of estk -> reciprocal tile.

    4 accumulators (2 on DVE, 2 on gpsimd), 4 heads each, then merge —
    bounded live tiles, moderate dependency depth."""
    engs = [nc.vector, nc.vector, nc.gpsimd, nc.gpsimd]
    accs = []
    for a in range(4):
        acc = elem.tile([128, 512], F32, tag=f"acc{a}")
        engs[a].tensor_tensor(acc[:], estk[:, 4 * a, :], estk[:, 4 * a + 1, :], ADD)
        engs[a].tensor_tensor(acc[:], acc[:], estk[:, 4 * a + 2, :], ADD)
        engs[a].tensor_tensor(acc[:], acc[:], estk[:, 4 * a + 3, :], ADD)
        accs.append(acc)
    nc.vector.tensor_tensor(accs[0][:], accs[0][:], accs[1][:], ADD)
    nc.gpsimd.tensor_tensor(accs[2][:], accs[2][:], accs[3][:], ADD)
    den = elem.tile([128, 512], F32, tag="den")
    nc.vector.tensor_tensor(den[:], accs[0][:], accs[2][:], ADD)
    rT = elem.tile([128, 512], F32, tag="rT")
    nc.vector.reciprocal(rT[:], den[:])
    return rT


def _build():
    MMDT = mybir.dt.float32r if USE_F32R else mybir.dt.float32
    nc = bacc.Bacc("TRN2", target_bir_lowering=False, debug=False,
                   num_devices=N_CORES)

    # ---- per-core external I/O ----
    qT = nc.dram_tensor("qT", [D, FQ], MMDT, kind="ExternalInput")
    kT = nc.dram_tensor("kT", [D, F], MMDT, kind="ExternalInput")
    vT = nc.dram_tensor("vT", [D, F], MMDT, kind="ExternalInput")
    WqT = nc.dram_tensor("WqT", [D, D], MMDT, kind="ExternalInput")
    WkT = nc.dram_tensor("WkT", [D, D], MMDT, kind="ExternalInput")
    WvT = nc.dram_tensor("WvT", [D, D], MMDT, kind="ExternalInput")
    W2T = nc.dram_tensor("W2T", [D, D], MMDT, kind="ExternalInput")
    bq32 = nc.dram_tensor("bq32", [D], F32, kind="ExternalInput")  # bq/32
    bk = nc.dram_tensor("bk", [D], F32, kind="ExternalInput")
    bv = nc.dram_tensor("bv", [D], F32, kind="ExternalInput")
    b2 = nc.dram_tensor("b2", [D], F32, kind="ExternalInput")
    attn_s = nc.dram_tensor("attn_s", [H, FQ, F], F32, kind="ExternalOutput")
    out_s = nc.dram_tensor("out_s", [2, FQ, D], F32, kind="ExternalOutput")

    # ---- internal DRAM ----
    vp_dram = nc.dram_tensor("vp_dram", [F // 128, 128, D], MMDT)
    res_loc = nc.dram_tensor("res_loc", [H, 64, FQ], MMDT)
    res_pair = nc.dram_tensor("res_pair", [2, H, 64, FQ], MMDT)

    def wview(t):  # [D, D] dram -> [128, 8, D] (e_in, e_out, col)
        return t.ap().rearrange("(eo ei) c -> ei eo c", ei=128)

    def tview(t):  # [D, N] dram -> [128, 8, N]
        return t.ap().rearrange("(eo ei) f -> ei eo f", ei=128)

    with tile.TileContext(nc) as tc:
        with (
            tc.tile_pool(name="biasc", bufs=1) as biasc,
            tc.tile_pool(name="ps", bufs=8, space="PSUM") as psum,
        ):
            bq_sb = biasc.tile([128, 8], F32, tag="bq")
            bk_sb = biasc.tile([128, 8], F32, tag="bk")
            bv_bc = biasc.tile([128, D], F32, tag="bv")
            b2_bc = biasc.tile([128, D], F32, tag="b2")
            nc.sync.dma_start(bq_sb[:], bq32.ap().rearrange("(co ci) -> ci co", ci=128))
            nc.sync.dma_start(bk_sb[:], bk.ap().rearrange("(co ci) -> ci co", ci=128))
            nc.sync.dma_start(bv_bc[:], bv.ap()[None, :].to_broadcast((128, D)))
            nc.sync.dma_start(b2_bc[:], b2.ap()[None, :].to_broadcast((128, D)))

            with tc.tile_pool(name="proj", bufs=1) as proj:   # lives S1..S5
                qpT_sb = proj.tile([128, 8, FQ], MMDT, tag="qpT")   # 16KB/par
                kpT_sb = proj.tile([128, 8, F], MMDT, tag="kpT")    # 64KB/par

                # ========= S1: q projection (transposed out, scaled) =========
                with tc.tile_pool(name="s1", bufs=1) as s1, \
                     tc.tile_pool(name="s1w", bufs=3) as s1w:
                    qT_sb = s1.tile([128, 8, FQ], MMDT, tag="qT")
                    nc.sync.dma_start(qT_sb[:], tview(qT))
                    for cc in range(8):
                        wq_c = s1w.tile([128, 8, 128], MMDT, tag="wchunk")
                        nc.sync.dma_start(wq_c[:], wview(WqT)[:, :, cc * 128:(cc + 1) * 128])
                        ps = psum.tile([128, FQ], F32, tag="ps")
                        for ei in range(8):
                            nc.tensor.matmul(ps[:], wq_c[:, ei, :], qT_sb[:, ei, :],
                                             start=(ei == 0), stop=(ei == 7))
                        nc.scalar.activation(qpT_sb[:, cc, :], ps[:], AF.Identity,
                                             bias=bq_sb[:, cc:cc + 1], scale=SCALE)

                # ========= S2: k projection (full batch, transposed out) =====
                with tc.tile_pool(name="s2", bufs=1) as s2, \
                     tc.tile_pool(name="s2k", bufs=2) as s2k:
                    WkT_sb = s2.tile([128, 8, D], MMDT, tag="wk")   # 32KB/par
                    nc.sync.dma_start(WkT_sb[:], wview(WkT))
                    for fkb in range(4):
                        kslab = s2k.tile([128, 8, 512], MMDT, tag="kslab")
                        nc.sync.dma_start(kslab[:], tview(kT)[:, :, fkb * 512:(fkb + 1) * 512])
                        for cc in range(8):
                            ps = psum.tile([128, 512], F32, tag="ps")
                            for ei in range(8):
                                nc.tensor.matmul(ps[:], WkT_sb[:, ei, cc * 128:(cc + 1) * 128],
                                                 kslab[:, ei, :],
                                                 start=(ei == 0), stop=(ei == 7))
                            nc.scalar.activation(kpT_sb[:, cc, fkb * 512:(fkb + 1) * 512],
                                                 ps[:], AF.Identity, bias=bk_sb[:, cc:cc + 1])

                # ========= S3: v projection (natural layout) -> DRAM =========
                with tc.tile_pool(name="s3", bufs=1) as s3, \
                     tc.tile_pool(name="s3v", bufs=3) as s3v:
                    WvT_sb = s3.tile([128, 8, D], MMDT, tag="wv")
                    nc.sync.dma_start(WvT_sb[:], wview(WvT))
                    for fc in range(F // 128):
                        vslab = s3v.tile([128, 8, 128], MMDT, tag="vslab")
                        nc.sync.dma_start(vslab[:], tview(vT)[:, :, fc * 128:(fc + 1) * 128])
                        for ch in range(2):
                            ps = psum.tile([128, 512], F32, tag="ps")
                            for ei in range(8):
                                nc.tensor.matmul(ps[:], vslab[:, ei, :],
                                                 WvT_sb[:, ei, ch * 512:(ch + 1) * 512],
                                                 start=(ei == 0), stop=(ei == 7))
                            vp_st = s3v.tile([128, 512], MMDT, tag="vpst")
                            nc.vector.tensor_tensor(vp_st[:], ps[:],
                                                    bv_bc[:, ch * 512:(ch + 1) * 512], ADD)
                            nc.sync.dma_start(vp_dram.ap()[fc, :, ch * 512:(ch + 1) * 512],
                                              vp_st[:])

                # ========= S4: pass B (transposed) + attn@v + res export ======
                with tc.tile_pool(name="resp", bufs=1) as resp:   # lives S4..S7
                    res_sb = resp.tile([64, H, FQ], F32, tag="res")  # 32KB/par
                    with tc.tile_pool(name="estkp", bufs=1) as estkp, \
                         tc.tile_pool(name="elem2", bufs=2) as elem2, \
                         tc.tile_pool(name="attns", bufs=4) as attns, \
                         tc.tile_pool(name="vpcp", bufs=2) as vpcp:

                        class ElemMux:
                            _n = 0
                            def tile(self, shape, dt, tag):
                                ElemMux._n += 1
                                return elem2.tile(shape, dt, tag=tag,
                                                  name=f"{tag}_{ElemMux._n}")
                        elem = ElemMux()

                        estk = estkp.tile([128, H, 512], F32, tag="estk")
                        for fkc in range(F // 128):
                            vp_c = vpcp.tile([128, D], MMDT, tag="vpc")
                            nc.sync.dma_start(vp_c[:], vp_dram.ap()[fkc])
                            for hp in range(8):
                                psa = psum.tile([128, 512], F32, tag="ps")
                                psb = psum.tile([128, 512], F32, tag="ps")
                                sl = slice(fkc * 128, (fkc + 1) * 128)
                                nc.tensor.matmul(psa[:], kpT_sb[0:64, hp, sl],
                                                 qpT_sb[0:64, hp, :], start=True, stop=True)
                                nc.tensor.matmul(psb[:], kpT_sb[64:128, hp, sl],
                                                 qpT_sb[64:128, hp, :], start=True, stop=True)
                                nc.scalar.activation(estk[:, 2 * hp, :], psa[:], AF.Exp)
                                nc.scalar.activation(estk[:, 2 * hp + 1, :], psb[:], AF.Exp)
                            rT = _denom_tree(nc, elem, estk)
                            for h in range(H):
                                at_t = attns.tile([128, 512], MMDT, tag="attnT")
                                eng = nc.vector if h % 2 == 0 else nc.gpsimd
                                eng.tensor_tensor(at_t[:], estk[:, h, :], rT[:], MULT)
                                psv = psum.tile([128, 512], F32, tag="ps")
                                nc.tensor.matmul(psv[0:64, :], vp_c[:, h * 64:(h + 1) * 64],
                                                 at_t[:], start=True, stop=True)
                                dst = res_sb[:, h, :]
                                if fkc == 0:
                                    nc.vector.tensor_copy(dst, psv[0:64, :])
                                else:
                                    nc.vector.tensor_tensor(dst, dst, psv[0:64, :], ADD)

                        # res export + pair AllGather (overlaps with pass A);
                        # gpsimd DMA casts f32 -> f32r (bit-identical)
                        nc.gpsimd.dma_start(res_loc.ap().rearrange("h p f -> p h f"),
                                            res_sb[:])
                        nc.gpsimd.collective_compute(
                            "AllGather", mybir.AluOpType.bypass,
                            replica_groups=[[0, 4], [1, 5], [2, 6], [3, 7]],
                            ins=[res_loc.ap().opt()], outs=[res_pair.ap().opt()],
                        )

                        # ========= S5: pass A (natural) -> attn output =========
                        for fqc in range(4):
                            for fkb in range(4):
                                for hp in range(8):
                                    psa = psum.tile([128, 512], F32, tag="ps")
                                    psb = psum.tile([128, 512], F32, tag="ps")
                                    qsl = slice(fqc * 128, (fqc + 1) * 128)
                                    ksl = slice(fkb * 512, (fkb + 1) * 512)
                                    nc.tensor.matmul(psa[:], qpT_sb[0:64, hp, qsl],
                                                     kpT_sb[0:64, hp, ksl],
                                                     start=True, stop=True)
                                    nc.tensor.matmul(psb[:], qpT_sb[64:128, hp, qsl],
                                                     kpT_sb[64:128, hp, ksl],
                                                     start=True, stop=True)
                                    nc.scalar.activation(estk[:, 2 * hp, :], psa[:], AF.Exp)
                                    nc.scalar.activation(estk[:, 2 * hp + 1, :], psb[:], AF.Exp)
                                rA = _denom_tree(nc, elem, estk)
                                for h in range(H):
                                    an_t = attns.tile([128, 512], F32, tag="attnA")
                                    eng = nc.vector if h % 2 == 0 else nc.gpsimd
                                    eng.tensor_tensor(an_t[:], estk[:, h, :], rA[:], MULT)
                                    nc.sync.dma_start(
                                        attn_s.ap()[h, fqc * 128:(fqc + 1) * 128,
                                                    fkb * 512:(fkb + 1) * 512], an_t[:])

                    # ========= S6/S7: output projection ======================
                    with tc.tile_pool(name="outp", bufs=1) as outp, \
                         tc.tile_pool(name="outw", bufs=3) as outw:
                        res2 = outp.tile([128, H, FQ], MMDT, tag="res2")
                        for bb in range(2):
                            nc.sync.dma_start(
                                res2[64 * bb:64 * bb + 64, :, :],
                                res_pair.ap()[bb].rearrange("h p f -> p h f"))
                        W2_sb = outp.tile([128, 8, D], MMDT, tag="w2")
                        nc.sync.dma_start(W2_sb[:], wview(W2T))
                        for hl in range(8):
                            for nb in range(2):
                                for bsel in range(2):
                                    h = hl + 8 * bsel
                                    ps = psum.tile([128, 512], F32, tag="ps")
                                    for cb in range(8):
                                        nc.tensor.matmul(
                                            ps[0:64, :], res2[:, h, cb:512:8],
                                            W2_sb[:, cb, nb * 512:(nb + 1) * 512],
                                            start=(cb == 0), stop=(cb == 7))
                                    out_t = outw.tile([64, 512], F32, tag="outt")
                                    nc.vector.tensor_tensor(
                                        out_t[:], ps[0:64, :],
                                        b2_bc[0:64, nb * 512:(nb + 1) * 512], ADD)
                                    nc.sync.dma_start(
                                        out_s.ap()[bsel, hl * 64:(hl + 1) * 64,
                                                   nb * 512:(nb + 1) * 512],
                                        out_t[:])

    nc.compile()
    return nc


def _get_nc():
    if "nc" not in _CACHE:
        _CACHE["nc"] = _build()
    return _CACHE["nc"]


def kernel(k, q, v, Wq, bq, Wk, bk, Wv, bv, W2, b2):
    k = np.asarray(k, np.float32)
    q = np.asarray(q, np.float32)
    v = np.asarray(v, np.float32)
    WqT = np.ascontiguousarray(np.asarray(Wq, np.float32).T)
    WkT = np.ascontiguousarray(np.asarray(Wk, np.float32).T)
    WvT = np.ascontiguousarray(np.asarray(Wv, np.float32).T)
    W2T = np.ascontiguousarray(np.asarray(W2, np.float32).T)
    bq32 = (np.asarray(bq, np.float32) * np.float32(SCALE)).astype(np.float32)
    bk_ = np.asarray(bk, np.float32)
    bv_ = np.asarray(bv, np.float32)
    b2_ = np.asarray(b2, np.float32)

    nc = _get_nc()
    in_maps = []
    for c in range(N_CORES):
        b, j = c // 4, c % 4
        in_maps.append({
            "qT": np.ascontiguousarray(q[b, j * FQ:(j + 1) * FQ, :].T),
            "kT": np.ascontiguousarray(k[b].T),
            "vT": np.ascontiguousarray(v[b].T),
            "WqT": WqT, "WkT": WkT, "WvT": WvT, "W2T": W2T,
            "bq32": bq32, "bk": bk_, "bv": bv_, "b2": b2_,
        })
    res = run_bass_kernel_spmd(nc, in_maps, core_ids=list(range(N_CORES)),
                               trace=False)

    attn = np.empty((B, H, F, F), np.float32)
    out = np.empty((B, F, D), np.float32)
    outv = out.reshape(B, 8, 4, 64, D)
    for c in range(N_CORES):
        b, j = c // 4, c % 4
        r = res.results[c]
        attn[b, :, j * FQ:(j + 1) * FQ, :] = r["attn_s"]
        outv[b, :, j] = r["out_s"][b].reshape(8, 64, D)
    return (out, attn)
